# revision 1
# baseline (speedup 1.0000x reference)
"""Trainium2 Bass kernel for nn_HVGuardModel (dense MoE routing).

Reference math (B=65536, D=1024, E=8, H=128, C1=64, NC=2):
    gw  = softmax(x @ Wg + bg)                      [B, E]
    h   = relu(einsum('bd,edh', x, We1) + be1)      [B, E, H]
    eo  = einsum('beh,eho', h, We2) + be2           [B, E, H]
    mix = einsum('be,beh', gw, eo)                  [B, H]
    out = relu(mix @ Wc1 + bc1) @ Wc2 + bc2         [B, NC]

Strategy: pure data-parallel over 8 cores (8192 rows each).  All device
activations live in "feature-major" layout [feature, batch] so the kernel
needs zero transposes -- the host supplies x pre-transposed (xT) and
transposes the [2, 8192] per-core outputs back.

Algebraic folds (host side):
  * mix is only consumed via mix @ Wc1  =>  fold V = We2 @ Wc1 per expert
    ([E*H, 64] stacked) and C = be2 @ Wc1; eo and mix are never materialized.
    This also fuses the gate mixing into one PSUM accumulation.
  * Layer-1 features are INTERLEAVED: f = j*E + e.  A "replicated gate"
    weight block (Wg columns tiled mod 8) yields a [128, N] logit tile whose
    row r holds logit[r mod 8] == the gate scale for row r of *every*
    h-block, so no cross-partition broadcast is ever needed.
  * softmax denominator: all-ones [8,128] lhsT matmul replicates
    s = sum_e exp across all 128 partitions; 1/s = Exp(-Ln(s)) on ACT
    (DVE reciprocal is ~8x slower per element; ACT Reciprocal is banned).
  * All biases are per-partition in this layout -> ride the ACT engine's
    native bias operand (out = f(in*scale + bias)); no bias matmuls.

Per 512-column batch tile: 83 matmuls (64 = the layer-1 grouped GEMM),
9 DVE ops, 13 ACT ops, 9 DMAs.  PE is the bottleneck engine.
"""

import numpy as np

B = 65536
D = 1024
E = 8
H = 128
C1 = 64
NCLS = 2
NCORES = 8
BLOC = B // NCORES  # 8192
NTILE = 512
F = E * H  # 1024
KD = D // 128  # 8 k-chunks over D
MH = F // 128  # 8 h-blocks
NMBLK = MH + 1  # + replicated-gate block

MM_DT = "float32r"  # matmul dtype: float32r | bfloat16

_BUILT = {}


def _np_store_dt(mm_dt_name):
    import ml_dtypes

    return np.float32 if mm_dt_name == "float32r" else ml_dtypes.bfloat16


def _build_nc(b_per_core: int, mm_dt_name: str, repeat: int = 1):
    """Build + compile the Bass module for one core (SPMD across 8).

    repeat > 1 wraps the whole batch loop in a hardware For_i loop that
    re-runs the identical work `repeat` times -- used only for timing
    (amortizes the ~45-90 ms axon dispatch/polling quantum away).
    """
    import concourse.bacc as bacc
    import concourse.tile as tile
    import concourse.mybir as mybir
    from contextlib import nullcontext

    nbt = b_per_core // NTILE
    fp32 = mybir.dt.float32
    # walrus requires fp32r matmul operands to be *produced* as fp32r, so all
    # PE-feeding tensors are declared in the matmul dtype end-to-end.
    st_dt = getattr(mybir.dt, mm_dt_name)

    def mm(ap):
        return ap

    nc = bacc.Bacc("TRN2", target_bir_lowering=False, debug=False)

    xT = nc.dram_tensor("xT", [D, b_per_core], st_dt, kind="ExternalInput")
    w1 = nc.dram_tensor("W1T", [128, NMBLK * KD * 128], st_dt, kind="ExternalInput")
    vb = nc.dram_tensor("Vb", [128, MH * C1], st_dt, kind="ExternalInput")
    s8 = nc.dram_tensor("S8", [8, C1 + 128], st_dt, kind="ExternalInput")
    wc2 = nc.dram_tensor("WC2", [C1, NCLS], st_dt, kind="ExternalInput")
    # per-partition bias columns (fp32): 0..7 = be1 block m, 8 = bg_rep,
    # 9 = bc1 (rows 0:64), 10 = bc2 (rows 0:2)
    bcol = nc.dram_tensor("BCOL", [128, 11], fp32, kind="ExternalInput")
    yT = nc.dram_tensor("yT", [NCLS, b_per_core], fp32, kind="ExternalOutput")

    AF = mybir.ActivationFunctionType
    OP = mybir.AluOpType

    with tile.TileContext(nc) as tc:
        with (
            tc.tile_pool(name="wpool", bufs=1) as wpool,
            tc.tile_pool(name="xpool", bufs=2) as xpool,
            tc.tile_pool(name="spool", bufs=2) as spool,
            tc.tile_pool(name="hpool", bufs=2) as hpool,
            tc.tile_pool(name="opool", bufs=2) as opool,
            tc.tile_pool(name="ps_gate", bufs=2, space="PSUM") as ps_gate,
            tc.tile_pool(name="ps_srep", bufs=1, space="PSUM") as ps_srep,
            tc.tile_pool(name="ps_h", bufs=2, space="PSUM") as ps_h,
            tc.tile_pool(name="ps_pre", bufs=1, space="PSUM") as ps_pre,
            tc.tile_pool(name="ps_out", bufs=2, space="PSUM") as ps_out,
        ):
            # ---- load weights/constants once ----
            # W1T split into per-m-block DMAs ordered by first use (gate
            # block first) so PE can start ~14us earlier than with one
            # monolithic 4.7MB transfer.
            w1t = wpool.tile([128, NMBLK * KD * 128], st_dt, tag="w1t")
            bct = wpool.tile([128, 11], fp32, tag="bct")
            s8t = wpool.tile([8, C1 + 128], st_dt, tag="s8t")
            vbt = wpool.tile([128, MH * C1], st_dt, tag="vbt")
            wc2t = wpool.tile([C1, NCLS], st_dt, tag="wc2t")
            def w1dma(m_):
                c0 = m_ * KD * 128
                nc.sync.dma_start(
                    w1t[:, c0 : c0 + KD * 128], w1[:, c0 : c0 + KD * 128]
                )

            def xdma(t):
                xk = []
                for k in range(KD):
                    xt_ = xpool.tile([128, NTILE], st_dt, tag=f"x{k}")
                    nc.sync.dma_start(
                        xt_[:],
                        xT[k * 128 : (k + 1) * 128, t * NTILE : (t + 1) * NTILE],
                    )
                    xk.append(xt_)
                return xk

            w1dma(MH)  # gate block first
            nc.sync.dma_start(bct[:], bcol[:])
            nc.sync.dma_start(s8t[:], s8[:])
            # btile-0 activations BEFORE the bulk weight blocks, so the first
            # gate matmuls are not queued behind 4.5MB of weight DMA.
            xk0 = xdma(0)
            for m_ in range(MH):
                w1dma(m_)
            nc.sync.dma_start(vbt[:], vb[:])
            nc.sync.dma_start(wc2t[:], wc2[:])

            def w1blk(m, k):
                c0 = (m * KD + k) * 128
                return w1t[:, c0 : c0 + 128]

            c_blk = s8t[:, 0:C1]  # [8, 64]   be2 @ Wc1
            ones8 = s8t[:, C1 : C1 + 128]  # [8, 128] ones

            rep_ctx = tc.For_i(0, repeat, 1) if repeat > 1 else nullcontext()
            with rep_ctx:
                _kernel_body(nc, tc, mybir, nbt, st_dt, mm, xpool, spool, hpool,
                             opool, ps_gate, ps_srep, ps_h, ps_pre, ps_out,
                             xT, yT, w1blk, c_blk, ones8, vbt, wc2t, bct,
                             xdma, xk0 if repeat == 1 else None)

    nc.compile()
    return nc


def _kernel_body(nc, tc, mybir, nbt, st_dt, mm, xpool, spool, hpool, opool,
                 ps_gate, ps_srep, ps_h, ps_pre, ps_out,
                 xT, yT, w1blk, c_blk, ones8, vbt, wc2t, bct, xdma, xk0):
    AF = mybir.ActivationFunctionType
    OP = mybir.AluOpType
    fp32 = mybir.dt.float32
    for t in range(nbt):
        b0 = t * NTILE
        # ---- load xT k-chunks (btile 0 may be pre-issued) ----
        xk = xk0 if (t == 0 and xk0 is not None) else xdma(t)

        # ---- replicated gate logits; exp(logit + bg) on ACT ----
        gp = ps_gate.tile([128, NTILE], fp32, tag="gate")
        for k in range(KD):
            nc.tensor.matmul(
                gp[:], mm(w1blk(MH, k)), mm(xk[k][:]),
                start=(k == 0), stop=(k == KD - 1),
            )
        expg = spool.tile([128, NTILE], st_dt, tag="expg")
        nc.scalar.activation(expg[:], gp[:], AF.Exp, bias=bct[:, 8:9])

        # ---- softmax denom, replicated; 1/s on DVE ----
        # (DVE reciprocal, NOT ACT Ln/Exp: keeping ACT's function mix to
        # {Exp, Relu, Identity} means one resident table set -- the per-set
        # LoadActFuncSet costs ~1.3us and stalled PE 1.6us every tile.)
        sp = ps_srep.tile([128, NTILE], fp32, tag="srep")
        nc.tensor.matmul(
            sp[:], mm(ones8), mm(expg[0:8, :]), start=True, stop=True
        )
        rinv = spool.tile([128, NTILE], fp32, tag="rinv")
        nc.vector.reciprocal(rinv[:], sp[:])

        # ---- normalized gate weights (replicated rows) ----
        gw = spool.tile([128, NTILE], st_dt, tag="gw")
        nc.vector.tensor_tensor(gw[:], expg[:], rinv[:], op=OP.mult)

        # ---- layer-1 h-blocks: relu(.+be1) on ACT, * gate on DVE ----
        hs = []
        for m in range(MH):
            hp = ps_h.tile([128, NTILE], fp32, tag="h")
            for k in range(KD):
                nc.tensor.matmul(
                    hp[:], mm(w1blk(m, k)), mm(xk[k][:]),
                    start=(k == 0), stop=(k == KD - 1),
                )
            hr = hpool.tile([128, NTILE], st_dt, tag=f"hs{m}")
            nc.scalar.activation(
                hr[:], hp[:], AF.Relu, bias=bct[:, m : m + 1]
            )
            nc.vector.tensor_tensor(hr[:], hr[:], gw[:], op=OP.mult)
            hs.append(hr)

        # ---- fused expert-2 + mix + cls-1: pre = V.T@hs + C.T@gw ----
        pp = ps_pre.tile([C1, NTILE], fp32, tag="pre")
        for k in range(MH):
            nc.tensor.matmul(
                pp[:], mm(vbt[:, k * C1 : (k + 1) * C1]), mm(hs[k][:]),
                start=(k == 0), stop=False,
            )
        nc.tensor.matmul(
            pp[:], mm(c_blk), mm(gw[0:8, :]), start=False, stop=True
        )
        rp = spool.tile([C1, NTILE], st_dt, tag="rp")
        nc.scalar.activation(
            rp[:], pp[:], AF.Relu, bias=bct[0:C1, 9:10]
        )

        # ---- cls-2: out = Wc2.T @ rp (+bc2 via ACT bias) ----
        op_ = ps_out.tile([NCLS, NTILE], fp32, tag="out")
        nc.tensor.matmul(op_[:], mm(wc2t[:]), mm(rp[:]), start=True, stop=True)
        ot = opool.tile([NCLS, NTILE], fp32, tag="o")
        nc.scalar.activation(
            ot[:], op_[:], AF.Identity, bias=bct[0:NCLS, 10:11]
        )
        nc.sync.dma_start(yT[0:NCLS, b0 : b0 + NTILE], ot[:])


def _get_nc(b_per_core: int, mm_dt_name: str, repeat: int = 1):
    key = (b_per_core, mm_dt_name, repeat)
    if key not in _BUILT:
        _BUILT[key] = _build_nc(b_per_core, mm_dt_name, repeat)
    return _BUILT[key]


def prep_inputs(x, We1, be1, We2, be2, Wg, bg, Wc1, bc1, Wc2, bc2,
                mm_dt_name=MM_DT, n_cores=NCORES):
    """Host-side packing -> list of per-core input maps."""
    f64 = np.float64
    sdt = _np_store_dt(mm_dt_name)
    b_per_core = x.shape[0] // n_cores

    # feature order f = j*E + e
    W1_all = np.transpose(np.asarray(We1, f64), (1, 2, 0)).reshape(D, F)
    Wg_rep = np.asarray(Wg, f64)[:, np.arange(128) % E]
    blocks = []
    for m_ in range(MH):
        for k in range(KD):
            blocks.append(W1_all[k * 128 : (k + 1) * 128, m_ * 128 : (m_ + 1) * 128])
    for k in range(KD):
        blocks.append(Wg_rep[k * 128 : (k + 1) * 128, :])
    W1T = np.ascontiguousarray(np.concatenate(blocks, axis=1).astype(sdt))

    V = np.einsum("ejk,kc->jec", np.asarray(We2, f64), np.asarray(Wc1, f64)).reshape(
        F, C1
    )
    Vb = np.ascontiguousarray(
        np.concatenate([V[k * 128 : (k + 1) * 128, :] for k in range(MH)], axis=1)
        .astype(sdt)
    )
    Cm = np.asarray(be2, f64) @ np.asarray(Wc1, f64)  # [E, C1]
    S8 = np.ascontiguousarray(
        np.concatenate([Cm, np.ones((E, 128), f64)], axis=1).astype(sdt)
    )
    WC2 = np.ascontiguousarray(np.asarray(Wc2, f64).astype(sdt))

    bcol = np.zeros((128, 11), np.float32)
    be1_int = np.asarray(be1, f64).T.reshape(F)  # f = j*E + e
    for m_ in range(MH):
        bcol[:, m_] = be1_int[m_ * 128 : (m_ + 1) * 128]
    bcol[:, 8] = np.asarray(bg, f64)[np.arange(128) % E]
    bcol[0:C1, 9] = np.asarray(bc1, f64)
    bcol[0:NCLS, 10] = np.asarray(bc2, f64)

    xT_full = np.ascontiguousarray(np.asarray(x).T.astype(sdt))  # [D, B]
    in_maps = []
    for c in range(n_cores):
        in_maps.append(
            {
                "xT": np.ascontiguousarray(
                    xT_full[:, c * b_per_core : (c + 1) * b_per_core]
                ),
                "W1T": W1T,
                "Vb": Vb,
                "S8": S8,
                "WC2": WC2,
                "BCOL": bcol,
            }
        )
    return in_maps, b_per_core


def run(inputs, mm_dt_name=MM_DT, trace=False):
    """Run on 8 NeuronCores; returns (y [B, 2] fp32, exec_time_ns or None)."""
    from concourse.bass_utils import run_bass_kernel_spmd

    in_maps, b_per_core = prep_inputs(**inputs, mm_dt_name=mm_dt_name)
    nc = _get_nc(b_per_core, mm_dt_name)
    res = run_bass_kernel_spmd(
        nc, in_maps, core_ids=list(range(NCORES)), trace=trace
    )
    y = np.concatenate([r["yT"].T for r in res.results], axis=0)
    return np.ascontiguousarray(y.astype(np.float32)), res.exec_time_ns


def kernel(**inputs):
    y, _ = run(inputs)
    return y

